# revision 1
# baseline (speedup 1.0000x reference)
"""nn_BayesianLayer — reparameterized Bayesian linear layer + inverted dropout
on 8 TRN2 NeuronCores (data-parallel over the 65536-row batch).

reference:
  w = w_mu + softplus(w_rho) * w_eps            [512, 512]
  b = b_mu + softplus(b_rho) * b_eps            [512]
  y = (x @ w.T + b) * (drop_u >= 0.2) / 0.8     [65536, 512]

Sharding: x and drop_u split into 8 row-shards of 8192; the small weight
tensors are replicated. Each core runs the same single-core Bass/Tile graph
(SPMD, no collectives); outputs are concatenated on the host.

Per-core kernel design:
 - x is fed host-transposed (xT [512, 8192]) because the TensorEngine
   contracts over the partition dim and fp32 DMA-transpose doesn't exist.
 - prologue computes w'T = 1.25*(w_mu + softplus(w_rho)*w_eps).T entirely
   on-device. softplus is relu(x) + ln1p(exp(-|x|)) with an 8-term
   polynomial for ln1p (this toolchain's ACT tables lack Softplus/Ln);
   the 1.25 dropout scale is folded into w', b'.
 - the bias is added via an extra K=1 matmul (ones[1,128].T @ b'[1,512])
   that initializes each PSUM accumulation group.
 - main loop: 8 groups of 1024 rows; per group one 2MB DMA each for
   xT/drop_u/y slabs; per 128-row tile 5 fp32r matmuls accumulate in one
   PSUM bank and a single fused DVE op applies the dropout mask:
   out = (drop_u >= 0.2) * psum.
 - matmul inputs are fp32r (TensorEngine fast-fp32 mode, 1 cycle/row at
   N=512 vs 4 for plain fp32); measured end-to-end rel err ~1.5e-4.
 - DMA issue is spread over three rings (x on SP, drop_u on GPSIMD/SWDGE,
   y on ACT) so loads and stores don't serialize on one descriptor ring.
"""

import numpy as np

import concourse.bass as bass
import concourse.mybir as mybir
from concourse import bacc, tile
from concourse.bass import ts
from concourse.bass_utils import run_bass_kernel_spmd

AF = mybir.ActivationFunctionType
ALU = mybir.AluOpType

N_CORES = 8
B, IN, OUT = 65536, 512, 512
BS = B // N_CORES          # 8192 rows per core
P = 128
KC = IN // P               # 4 contraction chunks
GROUPS = 8                 # batch groups per core
DROP = 0.2
SCALE = 1.0 / (1.0 - DROP)

# ln(1+t) ~= sum_{k=1..8} LN1P_COEF[k-1] * t^k on t in [0,1]  (max err 1.2e-7)
LN1P_COEF = [0.9999959, -0.49986133, 0.33169168, -0.24030304,
             0.1667245, -0.09422315, 0.035404634, -0.0062820404]


def build_kernel(x_bufs=2, du_bufs=3, out_bufs=3, psum_bufs=4,
                 x_engine="sync", du_engine="gpsimd", out_engine="scalar"):
    nc = bacc.Bacc(None, target_bir_lowering=False, debug=False)
    f32 = mybir.dt.float32
    f32r = mybir.dt.float32r
    gb = BS // GROUPS          # rows per group
    jt = gb // P               # output tiles per group

    xt = nc.declare_dram_parameter("xt", [IN, BS], f32, isOutput=False)
    wmu = nc.declare_dram_parameter("wmu", [IN, OUT], f32, isOutput=False)
    wrho = nc.declare_dram_parameter("wrho", [IN, OUT], f32, isOutput=False)
    weps = nc.declare_dram_parameter("weps", [IN, OUT], f32, isOutput=False)
    bmu = nc.declare_dram_parameter("bmu", [1, OUT], f32, isOutput=False)
    brho = nc.declare_dram_parameter("brho", [1, OUT], f32, isOutput=False)
    beps = nc.declare_dram_parameter("beps", [1, OUT], f32, isOutput=False)
    du = nc.declare_dram_parameter("du", [BS, OUT], f32, isOutput=False)
    y = nc.declare_dram_parameter("y", [BS, OUT], f32, isOutput=True)

    xt_r = xt[:, :].rearrange("(k p) b -> p k b", p=P)            # [128, KC, BS]
    wmu_r = wmu[:, :].rearrange("(k p) n -> p k n", p=P)          # [128, KC, OUT]
    wrho_r = wrho[:, :].rearrange("(k p) n -> p k n", p=P)
    weps_r = weps[:, :].rearrange("(k p) n -> p k n", p=P)
    du_r = du[:, :].rearrange("(g j p) n -> p g j n", p=P, j=jt)  # [128, G, jt, OUT]
    y_r = y[:, :].rearrange("(g j p) n -> p g j n", p=P, j=jt)

    with tile.TileContext(nc) as tc:
        with (
            tc.tile_pool(name="wt", bufs=1) as wt_pool,
            tc.tile_pool(name="prol", bufs=2) as prol_pool,
            tc.tile_pool(name="bias", bufs=1) as bias_pool,
            tc.tile_pool(name="xs", bufs=x_bufs) as x_pool,
            tc.tile_pool(name="dus", bufs=du_bufs) as du_pool,
            tc.tile_pool(name="outs", bufs=out_bufs) as out_pool,
            tc.tile_pool(name="ps", bufs=psum_bufs, space="PSUM") as psum_pool,
        ):
            def emit_softplus(sp, x_t, scratch):
                """sp = softplus(x_t) = relu(x) + ln1p(exp(-|x|))."""
                # scratch = exp(-|x|); |x| by clearing the sign bit (abs_max
                # is not in the DVE tensor_scalar ISA)
                nc.vector.tensor_scalar(
                    scratch[:].bitcast(mybir.dt.uint32),
                    x_t[:].bitcast(mybir.dt.uint32),
                    0x7FFFFFFF, None, ALU.bitwise_and)
                nc.scalar.activation(scratch[:], scratch[:], AF.Exp, scale=-1.0)
                # sp = poly(scratch): u = (u + a_k) * t, k = 8..1
                nc.vector.tensor_scalar_mul(sp[:], scratch[:], LN1P_COEF[-1])
                for a_k in reversed(LN1P_COEF[:-1]):
                    nc.vector.scalar_tensor_tensor(
                        sp[:], sp[:], a_k, scratch[:], ALU.add, ALU.mult)
                # scratch = relu(x); sp += scratch
                nc.scalar.activation(scratch[:], x_t[:], AF.Relu)
                nc.vector.tensor_add(sp[:], sp[:], scratch[:])

            # ---- weight prologue: w'T chunks [128, OUT], scaled by 1.25 ----
            wt = []
            for k in range(KC):
                mu_t = prol_pool.tile([P, OUT], f32, tag="mu")
                rho_t = prol_pool.tile([P, OUT], f32, tag="rho")
                eps_t = prol_pool.tile([P, OUT], f32, tag="eps")
                nc.sync.dma_start(out=mu_t[:], in_=wmu_r[:, k])
                nc.sync.dma_start(out=rho_t[:], in_=wrho_r[:, k])
                nc.sync.dma_start(out=eps_t[:], in_=weps_r[:, k])
                sp = prol_pool.tile([P, OUT], f32, tag="sp")
                scr = prol_pool.tile([P, OUT], f32, tag="scr")
                emit_softplus(sp, rho_t, scr)
                nc.vector.tensor_mul(sp[:], sp[:], eps_t[:])
                nc.vector.tensor_add(sp[:], sp[:], mu_t[:])
                wtk = wt_pool.tile([P, OUT], f32r, tag=f"wt{k}")
                nc.scalar.mul(wtk[:], sp[:], SCALE)
                wt.append(wtk)

            # ---- bias prologue: b' row [1, OUT], scaled by 1.25 ----
            bmu_t = bias_pool.tile([1, OUT], f32, tag="bmu")
            brho_t = bias_pool.tile([1, OUT], f32, tag="brho")
            beps_t = bias_pool.tile([1, OUT], f32, tag="beps")
            nc.sync.dma_start(out=bmu_t[:], in_=bmu[:, :])
            nc.sync.dma_start(out=brho_t[:], in_=brho[:, :])
            nc.sync.dma_start(out=beps_t[:], in_=beps[:, :])
            spb = bias_pool.tile([1, OUT], f32, tag="spb")
            scrb = bias_pool.tile([1, OUT], f32, tag="scrb")
            emit_softplus(spb, brho_t, scrb)
            nc.vector.tensor_mul(spb[:], spb[:], beps_t[:])
            nc.vector.tensor_add(spb[:], spb[:], bmu_t[:])
            b_row = bias_pool.tile([1, OUT], f32r, tag="brow")
            nc.scalar.mul(b_row[:], spb[:], SCALE)
            # memset can't write fp32r; go through an f32 tile + ACT copy
            ones_t = bias_pool.tile([1, P], f32r, tag="ones")
            ones_f = bias_pool.tile([1, P], f32, tag="onesf")
            nc.vector.memset(ones_f[:], 1.0)
            nc.scalar.copy(ones_t[:], ones_f[:])

            engines = {"sync": nc.sync, "scalar": nc.scalar, "gpsimd": nc.gpsimd}
            x_dma = engines[x_engine]
            du_dma = engines[du_engine]
            out_dma = engines[out_engine]

            # ---- main loop ----
            for g in range(GROUPS):
                xs = x_pool.tile([P, KC, gb], f32r, tag="xs")
                x_dma.dma_start(
                    out=xs[:], in_=xt_r[:, :, g * gb:(g + 1) * gb].bitcast(f32r))
                dus = du_pool.tile([P, jt, OUT], f32, tag="dus")
                du_dma.dma_start(out=dus[:], in_=du_r[:, g])
                outs = out_pool.tile([P, jt, OUT], f32, tag="outs")
                for j in range(jt):
                    ps = psum_pool.tile([P, OUT], f32, tag="ps")
                    nc.tensor.matmul(
                        ps[:], ones_t[:], b_row[:], start=True, stop=False)
                    for k in range(KC):
                        nc.tensor.matmul(
                            ps[:], xs[:, k, ts(j, P)], wt[k][:],
                            start=False, stop=(k == KC - 1))
                    # out = (drop_u >= 0.2) * psum   (one fused DVE op)
                    nc.vector.scalar_tensor_tensor(
                        outs[:, j], dus[:, j], DROP, ps[:], ALU.is_ge, ALU.mult)
                out_dma.dma_start(out=y_r[:, g], in_=outs[:])

    nc.finalize()
    return nc


def shard_inputs(x, w_mu, w_rho, b_mu, b_rho, w_eps, b_eps, drop_u):
    """Full inputs -> per-core in_maps (host-side slicing + layout prep)."""
    wmu_t = np.ascontiguousarray(np.asarray(w_mu, np.float32).T)
    wrho_t = np.ascontiguousarray(np.asarray(w_rho, np.float32).T)
    weps_t = np.ascontiguousarray(np.asarray(w_eps, np.float32).T)
    bmu = np.asarray(b_mu, np.float32).reshape(1, OUT)
    brho = np.asarray(b_rho, np.float32).reshape(1, OUT)
    beps = np.asarray(b_eps, np.float32).reshape(1, OUT)
    x = np.asarray(x, np.float32)
    drop_u = np.asarray(drop_u, np.float32)
    in_maps = []
    for c in range(N_CORES):
        sl = slice(c * BS, (c + 1) * BS)
        in_maps.append({
            "xt": np.ascontiguousarray(x[sl].T),
            "wmu": wmu_t, "wrho": wrho_t, "weps": weps_t,
            "bmu": bmu, "brho": brho, "beps": beps,
            "du": np.ascontiguousarray(drop_u[sl]),
        })
    return in_maps


def kernel(x, w_mu, w_rho, b_mu, b_rho, w_eps, b_eps, drop_u):
    nc = build_kernel()
    in_maps = shard_inputs(x, w_mu, w_rho, b_mu, b_rho, w_eps, b_eps, drop_u)
    res = run_bass_kernel_spmd(nc, in_maps, core_ids=list(range(N_CORES)))
    return np.ascontiguousarray(
        np.concatenate([res.results[c]["y"] for c in range(N_CORES)], axis=0))
